# revision 29
# baseline (speedup 1.0000x reference)
"""Trainium2 Bass kernel for the Clos-factorized MLP (nn_Clos_34282428956960).

The reference network
    h = x.reshape(b, c, 64, 64)                    # [b,c,n,r]
    h = einsum('bcnr,nrm->bcmr', h, w1) + bias1
    h = einsum('bcmr,rmn->bcnm', h, w2) + bias2
    h = einsum('bcnm,mro->bcor', h, w3) + bias3    # contracts BOTH n and m!
    y = h.reshape(b, c, -1)
collapses algebraically to a rank-256 linear map plus a constant row:

    G = X @ W1f           X: [T,4096], W1f[d=(n,r), m] = w1[n,r,m]*w2s[r,m]
    Y = G @ W3f + crow    W3f[m, o*64+r] = w3[m,r,o]
    crow = (bias1@w2s + 64*bias2) @ W3f + tile(bias3)   (constant [4096] row)

Device kernel (per core, tokens sharded 8 ways):
  - x is transposed + cast to bf16 on the HOST (input marshalling), so the
    device receives X^T tiles [128d, t] directly: no on-chip transposes.
  - MM1: G^T[m,t] += W1f[d,m].T @ X^T[d,t], 32 d-tiles streamed kt-by-kt
    into 4 PSUM banks (2 m-tiles x 2 token-halves of 512).
  - G^T copied PSUM->SBUF as bf16 (DVE/ACT alternating).
  - MM2: Y[t,j] += G^T[m,t].T @ W3f[m,j], accumulating 2 m-tiles, j in 8
    tiles of 512; PSUM->SBUF bf16 copies alternate DVE/ACT; 256KB stores.
  - crow (all biases) is added on the host during the bf16->f32 upcast.
  - A few warm-up/bridge matmuls keep the PE clock-gate (HAM) warm across
    the DMA-latency prefix and the MM1->MM2 handoff.
"""

import numpy as np
import ml_dtypes

TOK_TOTAL = 8192          # b*c = 2*4096 tokens
N_CORES = 8
TOK = TOK_TOTAL // N_CORES  # 1024 tokens per core
D = 4096                  # input features
M = 256                   # bottleneck
J = 4096                  # output features
KT = D // 128             # 32 d-tiles
MT = M // 128             # 2 m-tiles
TH = TOK // 512           # 2 token halves for MM1 (N=512 each)
NTT = TOK // 128          # 8 token tiles for MM2
JT = 512                  # output column tile (one PSUM bank)
NJ = J // JT              # 8 j-tiles

# x/w1 DMA group sizes in kt units (schedule-tuned against TimelineSim)
XG_SIZES = [3] + [2] * 14 + [1]
N_WARM = 4                # PE warm-up matmuls (N=256) before MM1
N_BRIDGE = 4              # PE bridge matmuls between MM1 and MM2

_CACHE = {}


def _build_nc():
    import concourse.mybir as mybir
    import concourse.tile as tile
    from concourse import bacc

    F32 = mybir.dt.float32
    BF16 = mybir.dt.bfloat16

    nc = bacc.Bacc("TRN2", target_bir_lowering=False, debug=False,
                   num_devices=N_CORES)
    xt = nc.dram_tensor("xt", [128, KT, TOK], BF16, kind="ExternalInput")
    w1t = nc.dram_tensor("w1t", [128, KT, M], BF16, kind="ExternalInput")
    w3t = nc.dram_tensor("w3t", [128, MT, J], BF16, kind="ExternalInput")
    y = nc.dram_tensor("y", [TOK, J], BF16, kind="ExternalOutput")

    with tile.TileContext(nc) as tc:
        with (
            tc.tile_pool(name="const", bufs=1) as const_pool,
            tc.tile_pool(name="yout", bufs=8) as yout_pool,
            tc.tile_pool(name="g_psum", bufs=1, space="PSUM") as g_psum,
            tc.tile_pool(name="y_psum", bufs=1, space="PSUM") as y_psum,
        ):
            w1_sb = const_pool.tile([128, KT, M], BF16)
            xt_sb = const_pool.tile([128, KT, TOK], BF16)
            w3_sb = const_pool.tile([128, MT, J], BF16)
            gt = const_pool.tile([128, MT, TOK], BF16)

            # ---- DMA program order: w1 tranches finely interleaved with
            # same-kt x groups so MM1 is fed just-in-time; w3 streams last in
            # 8 slices that MM2's j-quarter-outer loop consumes as they land.
            sizes = XG_SIZES
            assert sum(sizes) == KT
            xg = []
            k0 = 0
            for nk in sizes:
                xg.append((k0, nk))
                k0 += nk
            for k0, nk in xg:
                nc.sync.dma_start(w1_sb[:, k0:k0 + nk, :],
                                  w1t[:, k0:k0 + nk, :])
                nc.sync.dma_start(xt_sb[:, k0:k0 + nk, :],
                                  xt[:, k0:k0 + nk, :])
            for s in range(8):
                nc.sync.dma_start(w3_sb[:, :, 512 * s:512 * (s + 1)],
                                  w3t[:, :, 512 * s:512 * (s + 1)])

            # ---- MM1: stream kt; 4 concurrent PSUM accumulations.
            # A few streamed warm-up matmuls (rotating banks, no WAW chains)
            # bridge the DMA-latency prefix and ramp the PE clock-gate;
            # kt==0's start=True overwrites their garbage.
            gps = [[g_psum.tile([128, 512], F32, name=f"gp{mt}_{th}")
                    for th in range(TH)] for mt in range(MT)]
            for i in range(N_WARM):
                nc.tensor.matmul(gps[(i // 2) % 2][i % 2][:, :256],
                                 w1_sb[:, 0, :128], w1_sb[:, 0, :],
                                 start=True, stop=True)
            for kt in range(KT):
                for mt in range(MT):
                    for th in range(TH):
                        nc.tensor.matmul(
                            gps[mt][th][:],
                            w1_sb[:, kt, mt * 128:(mt + 1) * 128],
                            xt_sb[:, kt, th * 512:(th + 1) * 512],
                            start=(kt == 0), stop=(kt == KT - 1))

            # ---- G^T PSUM->SBUF (bf16), both engines in parallel.
            cp = 0
            for th in range(TH):
                for mt in range(MT):
                    dst = gt[:, mt, th * 512:(th + 1) * 512]
                    if cp % 2 == 0:
                        nc.vector.tensor_copy(dst, gps[mt][th][:])
                    else:
                        nc.scalar.copy(dst, gps[mt][th][:])
                    cp += 1

            # ---- bridge matmuls: keep PE busy across the G copies and the
            # w3 DMA tail (writes land in y_psum slots, overwritten later).
            # MM2 rotates over the 4 y_psum banks plus the 4 MM1 banks
            # (reusable once their G copy has drained).
            ypool = [y_psum.tile([128, JT], F32, name=f"yp{i}")
                     for i in range(4)]
            ypool += [gps[mt][th] for mt in range(MT) for th in range(TH)]
            for i in range(N_BRIDGE):
                nc.tensor.matmul(ypool[i % 4][:, :256],
                                 w1_sb[:, 2, :128], w1_sb[:, 3, :],
                                 start=True, stop=True)

            # ---- MM2 + store: j-quarter outer so MM2 starts as soon as
            # the first w3 slices land; [128,1024] (256KB) stores.
            yi = 0
            for jq in range(4):
                for tt in range(NTT):
                    yo = yout_pool.tile([128, 1024], BF16, name="yo")
                    for j2 in range(2):
                        jt = jq * 2 + j2
                        yp = ypool[yi % len(ypool)]
                        yi += 1
                        for mt in range(MT):
                            nc.tensor.matmul(
                                yp[:],
                                gt[:, mt, tt * 128:(tt + 1) * 128],
                                w3_sb[:, mt, jt * JT:(jt + 1) * JT],
                                start=(mt == 0), stop=(mt == MT - 1))
                        dst = yo[:, j2 * JT:(j2 + 1) * JT]
                        if cp % 2 == 0:
                            nc.vector.tensor_copy(dst, yp[:])
                        else:
                            nc.scalar.copy(dst, yp[:])
                        cp += 1
                    nc.sync.dma_start(
                        y[tt * 128:(tt + 1) * 128,
                          jq * 1024:(jq + 1) * 1024], yo[:])
    nc.compile()
    return nc


def _fold_weights(w1, w2, w3, bias1, bias2, bias3):
    """Collapse the 3-stage Clos into W1f [4096,256], W3f [256,4096], crow."""
    w1 = np.asarray(w1, np.float64)
    w2 = np.asarray(w2, np.float64)
    w3 = np.asarray(w3, np.float64)
    b1 = np.asarray(bias1, np.float64)
    b2 = np.asarray(bias2, np.float64)
    b3 = np.asarray(bias3, np.float64)

    w2s = w2.sum(axis=2)                                   # [64(r), 256(m)]
    W1f = (w1 * w2s[None, :, :]).reshape(D, M)             # [(n,r), m]
    c2 = b1 @ w2s + w2.shape[2] * b2                       # [256]
    W3f = np.transpose(w3, (0, 2, 1)).reshape(M, J)        # [m, (o,r)]
    c3 = np.tile(b3, J // b3.shape[0])                     # [4096], period 64
    crow = c2 @ W3f + c3                                   # constant output row
    return W1f, W3f, crow


def _device_consts(w1, w2, w3, bias1, bias2, bias3):
    W1f, W3f, crow = _fold_weights(w1, w2, w3, bias1, bias2, bias3)
    bf16 = ml_dtypes.bfloat16
    w1t = np.ascontiguousarray(
        W1f.reshape(KT, 128, M).transpose(1, 0, 2)).astype(bf16)
    w3t = np.ascontiguousarray(
        W3f.reshape(MT, 128, J).transpose(1, 0, 2)).astype(bf16)
    return {"w1t": w1t, "w3t": w3t}, crow.astype(np.float32)


def _shard_x(x):
    """Full x [B,C,D] fp32 -> per-core transposed bf16 [128, KT, TOK]."""
    bf16 = ml_dtypes.bfloat16
    x2d = np.asarray(x, np.float32).reshape(TOK_TOTAL, D)
    shards = []
    for i in range(N_CORES):
        xc = x2d[i * TOK:(i + 1) * TOK]                    # [TOK, D]
        # xt[p, kt, t] = xc[t, kt*128 + p]
        xt = np.ascontiguousarray(
            xc.T.reshape(KT, 128, TOK).transpose(1, 0, 2)).astype(bf16)
        shards.append(xt)
    return shards


def _make_in_maps(x, w1, w2, w3, bias1, bias2, bias3):
    consts, crow = _device_consts(w1, w2, w3, bias1, bias2, bias3)
    shards = _shard_x(x)
    in_maps = [{"xt": shards[i], **consts} for i in range(N_CORES)]
    return in_maps, crow


def kernel(x, w1, w2, w3, bias1, bias2, bias3):
    from concourse.bass_utils import run_bass_kernel_spmd

    in_maps, crow = _make_in_maps(x, w1, w2, w3, bias1, bias2, bias3)

    if "nc" not in _CACHE:
        _CACHE["nc"] = _build_nc()
    nc = _CACHE["nc"]

    res = run_bass_kernel_spmd(nc, in_maps, core_ids=list(range(N_CORES)))
    y = np.concatenate(
        [np.asarray(res.results[i]["y"]) for i in range(N_CORES)], axis=0)
    y = y.astype(np.float32) + crow[None, :]
    return y.reshape(x.shape[0], x.shape[1], J)
